# revision 28
# baseline (speedup 1.0000x reference)
"""Additive (Bahdanau) attention on 8 TRN2 NeuronCores (raw Bass).

Reference math (B=4, Tq=256, Tk=512, Dq=Dv=512, U=256):
    q = query @ W1; k = value @ W2
    scores[t,s] = sum_u scale[u] * tanh(q[t,u] + k[s,u])
    attn = softmax(scores, -1); context = attn @ value

Separable-sine reformulation: fit  tanh(z) ~= sum_m b_m sin(w_m z)
(M=8 free frequencies), then sin(w(q+k)) = sin(wq)cos(wk)+cos(wq)sin(wk):
    scores ~= sum_m (b_m scale_u sin(w_m q)) @ cos(w_m k)^T
            + (b_m scale_u cos(w_m q)) @ sin(w_m k)^T
i.e. 2M=16 rank-U matmuls.  The O(Tq*Tk*U) tanh tensor is never formed:
ACT evaluates sin only on the small q ([128,256]) / k ([512,256])
matrices.

The device Sin spline is accurate only for |arg| <~ 3.5, so arguments
are range-reduced per mode with a 2-op fp32 bit trick on DVE:
    u = z*(w/2pi) + 1536.625          (ts mult,add; exponent pinned
                                       to 2^10 so low 13 mantissa bits
                                       hold frac(u) * 2^13)
    w32 = (u & 0x1FFF) | 0x3F800000   (ts and,or; w32 in [1,2))
then the ACT's exact-FMA affine maps it back:
    sin(w z)  = Sin(2048pi * w32 - 2048pi - 5pi/4)
    cos(w z)  = Sin(... + pi/2)        args in [-pi-pi/4, pi-pi/4].

Softmax runs in [t_p, s] layout: exp with accum_out produces row sums
for free; attn needs no transpose; context uses 4 PE transposes of E.
Input DMAs are spread over all five engine queues (per-queue DMA
bandwidth ~45GB/s is the startup bottleneck).

Sharding: (b, tq-half) -> 8 cores, 128 query rows each; Tk local.
"""

from contextlib import ExitStack

import numpy as np

import concourse.bass as bass
import concourse.mybir as mybir
from concourse.bass_utils import run_bass_kernel_spmd

F32 = mybir.dt.float32
I32 = mybir.dt.int32
BF16 = mybir.dt.bfloat16
AF = mybir.ActivationFunctionType
OP = mybir.AluOpType

N_CORES = 8
B, TQ, TK, DQ, DV, U = 4, 256, 512, 512, 512, 256
T_ROWS = 128
UC = U // 128          # 2
DC = DQ // 128         # 4
SC = TK // 128         # 4
M = 7                  # sine modes
H = 2                  # mode halves
QH = [(0, 4), (4, 3)]            # q-side half groups (m0, nm)
KQ = [(0, 2), (2, 2), (4, 2), (6, 1)]  # k-side quarter groups

WS = [0.25127647, 0.75689382, 1.27079933, 1.79612467,
      2.32596714, 2.92463035, 3.96080371]
BS = [1.24210203, 0.34188005, 0.14334471, 0.06416025,
      0.02653623, 0.01565173, 0.00512583]

SC2 = float(np.float32(1024 * 2 * np.pi))
BIAS_S = float(np.float32(-np.float64(np.float32(SC2)) - np.pi - np.pi / 4))
BIAS_C = float(np.float32(-np.float64(np.float32(SC2)) - np.pi + np.pi / 4))
OFFS = 1536.625


def build_bass() -> bass.Bass:
    nc = bass.Bass()
    qt_ext = nc.declare_dram_parameter("queryT", [128, DC * 128], BF16, isOutput=False)
    vt_ext = nc.declare_dram_parameter("valueT", [128, DC * TK], BF16, isOutput=False)
    vb_ext = nc.declare_dram_parameter("valuebf", [128, SC * DV], BF16, isOutput=False)
    w1_ext = nc.declare_dram_parameter("W1b", [128, DC * U], BF16, isOutput=False)
    w2_ext = nc.declare_dram_parameter("W2b", [128, DC * U], BF16, isOutput=False)
    bst_ext = nc.declare_dram_parameter("bstab", [128, M * UC * 128], BF16, isOutput=False)
    idb_ext = nc.declare_dram_parameter("identb", [128, 128], BF16, isOutput=False)
    ctx_ext = nc.declare_dram_parameter("context", [T_ROWS, DV], F32, isOutput=True)
    attn_ext = nc.declare_dram_parameter("attn", [T_ROWS, TK], F32, isOutput=True)

    es = ExitStack()
    with es:
        _n = [0]

        def sb(shape, dt):
            _n[0] += 1
            return es.enter_context(nc.sbuf_tensor(f"sb{_n[0]}", shape, dt))

        # ---- SBUF ----
        vTb = sb([128, DC * TK], BF16)          # [d_p, (dc, s)]
        qTb = sb([128, DC * 128], BF16)         # [d_p, (dc, t)]
        w1b = sb([128, DC * U], BF16)
        w2b = sb([128, DC * U], BF16)
        v_bf = sb([128, SC * DV], BF16)         # [s_p, (sc, d)]
        bs_full = sb([128, M * UC * 128], BF16)  # [u_p, (m, uc, t-bcast)]
        ident_bf = sb([128, 128], BF16)
        q_f = sb([128, UC * 128], F32)          # [u_p, (uc, t)]
        k_f = sb([128, UC * TK], F32)           # [u_p, (uc, s)]
        u_q = sb([128, M * UC * 128], F32)      # [u_p, (m, uc, t)]
        w_q = sb([128, M * UC * 128], F32)
        u_k = sb([128, M * UC * TK], F32)       # [u_p, (m, uc, s)]
        w_k = sb([128, M * UC * TK], F32)
        Sq = sb([128, M * UC * 128], BF16)
        Cq = sb([128, M * UC * 128], BF16)
        SqF = sb([128, M * UC * 128], BF16)     # folded with b_m*scale_u
        CqF = sb([128, M * UC * 128], BF16)
        Sk = sb([128, M * UC * TK], BF16)
        Ck = sb([128, M * UC * TK], BF16)
        E_sb = sb([128, TK], BF16)              # [t_p, s]
        ET_sb = sb([128, SC * 128], BF16)       # [s_p, (sc, t)]
        sums = sb([128, 1], F32)
        r_sb = sb([128, 1], F32)
        attn_sb = sb([128, TK], F32)
        ctx_sb = sb([128, DV], F32)
        bias_s = sb([128, 1], F32)
        bias_c = sb([128, 1], F32)
        scratch = sb([128, 1], F32)

        QW = UC * 128        # 256 free elems per mode, q side
        KW = UC * TK         # 1024 per mode, k side

        # ---- PSUM ----
        psA = es.enter_context(nc.psum_tensor("psA", [128, 2048], F32))
        psB = es.enter_context(nc.psum_tensor("psB", [128, 2048], F32))
        scores_ps = psA[:, 0:512]
        ctx_ps = psA[:, 512:1024]
        tra_ps = psA[:, 1024:1536]
        k_ps = [psB[:, 0:512], psB[:, 512:1024]]
        q_ps = [psB[:, 1024:1152], psB[:, 1536:1664]]
        q_ps_view = psB[:, 1024:2048].rearrange("p (uc x) -> p uc x", uc=2)[:, :, 0:128]
        tra_bf = tra_ps.bitcast(BF16)           # [128, 1024] bf16

        sem = lambda name: es.enter_context(nc.semaphore(name))
        s_qt = sem("s_qt")
        s_w1a = sem("s_w1a")
        s_w1b = sem("s_w1b")
        s_w2a = sem("s_w2a")
        s_w2b = sem("s_w2b")
        s_vt = [sem(f"s_vt{i}") for i in range(DC)]
        s_vbf = sem("s_vbf")
        s_idb = sem("s_idb")
        s_bst = sem("s_bst")
        s_c = sem("s_c")
        s_proj = sem("s_proj")   # q0,q1,k0,k1
        s_evq = sem("s_evq")
        s_evk = sem("s_evk")
        s_uq = sem("s_uq")
        s_uk = sem("s_uk")
        s_yq = sem("s_yq")       # w_q halves ready
        s_yk = sem("s_yk")
        s_trig = sem("s_trig")   # qh0 s,c qh1 s,c kh0 s,c kh1 s,c
        s_fold = sem("s_fold")   # h0 S,C h1 S,C
        s_mm = sem("s_mm")
        s_exp = sem("s_exp")
        s_tra = sem("s_tra")
        s_evt = sem("s_evt")
        s_ctx = sem("s_ctx")
        s_o = sem("s_o")
        s_dout = sem("s_dout")
        s_dout2 = sem("s_dout2")

        def ts1(vector, out_t, in_t, m, width):
            return vector.tensor_scalar(
                out=out_t[:, m * width : (m + 1) * width],
                in0=in_t[:, :],
                scalar1=float(WS[m] / (2 * np.pi)),
                scalar2=OFFS,
                op0=OP.mult,
                op1=OP.add,
            )

        def ts2(vector, out_t, in_t, m0, nm, width):
            sl = slice(m0 * width, (m0 + nm) * width)
            return vector.tensor_scalar(
                out=out_t[:, sl].bitcast(I32),
                in0=in_t[:, sl].bitcast(I32),
                scalar1=0x00001FFF,
                scalar2=0x3F800000,
                op0=OP.bitwise_and,
                op1=OP.bitwise_or,
            )

        with nc.Block() as block:

            @block.sync
            def _(sync):
                sync.dma_start(out=qTb[:, :], in_=qt_ext[:, :]).then_inc(s_qt, 16)
                sync.dma_start(
                    out=vTb[:, 0:TK], in_=vt_ext[:, 0:TK]
                ).then_inc(s_vt[0], 16)
                sync.dma_start(
                    out=vTb[:, 1 * TK : 2 * TK], in_=vt_ext[:, 1 * TK : 2 * TK]
                ).then_inc(s_vt[1], 16)
                sync.wait_ge(s_o, 1)
                sync.dma_start(out=attn_ext[:, 0:256], in_=attn_sb[:, 0:256]).then_inc(s_dout, 16)
                sync.wait_ge(s_o, 2)
                sync.dma_start(out=ctx_ext[:, 256:512], in_=ctx_sb[:, 256:512]).then_inc(s_dout, 16)
                sync.wait_ge(s_dout, 48)
                sync.wait_ge(s_dout2, 16)

            @block.gpsimd
            def _(gpsimd):
                gpsimd.dma_start(
                    out=w1b[:, 2 * U : 4 * U], in_=w1_ext[:, 2 * U : 4 * U]
                ).then_inc(s_w1b, 16)
                gpsimd.dma_start(
                    out=w2b[:, 2 * U : 4 * U], in_=w2_ext[:, 2 * U : 4 * U]
                ).then_inc(s_w2b, 16)
                gpsimd.dma_start(
                    out=vTb[:, 3 * TK : 4 * TK], in_=vt_ext[:, 3 * TK : 4 * TK]
                ).then_inc(s_vt[3], 16)
                gpsimd.dma_start(out=bs_full[:, :], in_=bst_ext[:, :]).then_inc(s_bst, 16)
                gpsimd.dma_start(out=v_bf[:, :], in_=vb_ext[:, :]).then_inc(s_vbf, 16)
                gpsimd.dma_start(out=ident_bf[:, :], in_=idb_ext[:, :]).then_inc(s_idb, 16)
                gpsimd.wait_ge(s_o, 2)
                gpsimd.dma_start(out=ctx_ext[:, 0:256], in_=ctx_sb[:, 0:256]).then_inc(s_dout2, 16)

            @block.vector
            def _(vector):
                vector.memset(bias_s[:, :], BIAS_S)
                vector.memset(bias_c[:, :], BIAS_C).then_inc(s_c, 1)
                # q reductions, per half: ts1 x4 then ts2 (self-sems order
                # same-engine RAW for the race model; ~free on the queue)
                vector.wait_ge(s_evq, 1)
                for h, (m0, nm) in enumerate(QH):
                    for ml in range(nm):
                        ins = ts1(vector, u_q, q_f, m0 + ml, QW)
                    ins.then_inc(s_uq, 1)
                    vector.wait_ge(s_uq, h + 1)
                    ts2(vector, w_q, u_q, m0, nm, QW).then_inc(s_yq, 1)
                # k reductions, quarter-granular
                vector.wait_ge(s_evk, 1)
                for qt, (m0, nm) in enumerate(KQ):
                    for ml in range(nm):
                        ins = ts1(vector, u_k, k_f, m0 + ml, KW)
                    ins.then_inc(s_uk, 1)
                    vector.wait_ge(s_uk, qt + 1)
                    ts2(vector, w_k, u_k, m0, nm, KW).then_inc(s_yk, 1)
                # folds: SqF/CqF = Sq/Cq * (b_m scale_u), full-size table
                vector.wait_ge(s_bst, 16)
                for h, (m0, nm) in enumerate(QH):
                    vector.wait_ge(s_trig, 2 * h + 2)
                    sl = slice(m0 * QW, (m0 + nm) * QW)
                    for src, dst in ((Sq, SqF), (Cq, CqF)):
                        vector.tensor_tensor(
                            out=dst[:, sl], in0=src[:, sl], in1=bs_full[:, sl],
                            op=OP.mult,
                        ).then_inc(s_fold, 1)
                # epilogue
                vector.wait_ge(s_exp, 1)
                vector.reciprocal(out=r_sb[:, :], in_=sums[:, :])
                vector.drain()
                vector.tensor_scalar_mul(
                    out=attn_sb[:, :], in0=E_sb[:, :], scalar1=r_sb[:, 0:1]
                ).then_inc(s_o, 1)
                vector.wait_ge(s_ctx, 1)
                vector.tensor_scalar_mul(
                    out=ctx_sb[:, :], in0=ctx_ps, scalar1=r_sb[:, 0:1]
                ).then_inc(s_o, 1)

            @block.scalar
            def _(scalar):
                scalar.dma_start(
                    out=w1b[:, 0 : 2 * U], in_=w1_ext[:, 0 : 2 * U]
                ).then_inc(s_w1a, 16)
                scalar.dma_start(
                    out=w2b[:, 0 : 2 * U], in_=w2_ext[:, 0 : 2 * U]
                ).then_inc(s_w2a, 16)
                scalar.dma_start(
                    out=vTb[:, 2 * TK : 3 * TK], in_=vt_ext[:, 2 * TK : 3 * TK]
                ).then_inc(s_vt[2], 16)
                # dummy sin pulls the trig table load off the critical path
                scalar.wait_ge(s_c, 1)
                scalar.activation(out=scratch[:, :], in_=bias_s[:, :], func=AF.Sin)
                # q evac
                scalar.wait_ge(s_proj, 2)
                scalar.copy(
                    out=q_f[:, :].rearrange("p (uc t) -> p uc t", uc=2),
                    in_=q_ps_view,
                ).then_inc(s_evq, 1)
                # q trig
                for h, (m0, nm) in enumerate(QH):
                    qs = slice(m0 * QW, (m0 + nm) * QW)
                    scalar.wait_ge(s_yq, h + 1)
                    scalar.activation(out=Sq[:, qs], in_=w_q[:, qs], func=AF.Sin,
                                      scale=SC2, bias=bias_s[:, 0:1]).then_inc(s_trig, 1)
                    scalar.activation(out=Cq[:, qs], in_=w_q[:, qs], func=AF.Sin,
                                      scale=SC2, bias=bias_c[:, 0:1]).then_inc(s_trig, 1)
                # k evac
                scalar.wait_ge(s_proj, 4)
                scalar.copy(out=k_f[:, :], in_=psB[:, 0:1024]).then_inc(s_evk, 1)
                # k trig, quarter-granular
                for qt, (m0, nm) in enumerate(KQ):
                    ks = slice(m0 * KW, (m0 + nm) * KW)
                    scalar.wait_ge(s_yk, qt + 1)
                    scalar.activation(out=Sk[:, ks], in_=w_k[:, ks], func=AF.Sin,
                                      scale=SC2, bias=bias_s[:, 0:1]).then_inc(s_trig, 1)
                    scalar.activation(out=Ck[:, ks], in_=w_k[:, ks], func=AF.Sin,
                                      scale=SC2, bias=bias_c[:, 0:1]).then_inc(s_trig, 1)
                # dummy exp: pull the exp table load off the critical path
                scalar.activation(out=scratch[:, :], in_=bias_s[:, :], func=AF.Exp)
                # softmax exp with free row sums
                scalar.wait_ge(s_mm, 1)
                scalar.activation(out=E_sb[:, :], in_=scores_ps, func=AF.Exp,
                                  accum_out=sums[:, 0:1]).then_inc(s_exp, 1)
                # ET evac for the context matmuls
                scalar.wait_ge(s_tra, 4)
                scalar.copy(out=ET_sb[:, :], in_=tra_bf[:, 0 : SC * 128]).then_inc(s_evt, 1)
                # attn second half on this queue
                scalar.wait_ge(s_o, 1)
                scalar.dma_start(out=attn_ext[:, 256:512], in_=attn_sb[:, 256:512]).then_inc(s_dout, 16)

            @block.tensor
            def _(tensor):
                # q projection, dc-pipelined
                tensor.wait_ge(s_qt, 16)
                for dc in range(DC):
                    tensor.wait_ge(s_w1a if dc < 2 else s_w1b, 16)
                    for uc in range(UC):
                        ins = tensor.matmul(
                            out=q_ps[uc],
                            lhsT=w1b[:, dc * U + uc * 128 : dc * U + uc * 128 + 128],
                            rhs=qTb[:, dc * 128 : (dc + 1) * 128],
                            start=(dc == 0),
                            stop=(dc == DC - 1),
                        )
                        if dc == DC - 1:
                            ins.then_inc(s_proj, 1)
                # k projection, dc-pipelined
                for dc in range(DC):
                    tensor.wait_ge(s_w2a if dc < 2 else s_w2b, 16)
                    tensor.wait_ge(s_vt[dc], 16)
                    for uc in range(UC):
                        ins = tensor.matmul(
                            out=k_ps[uc],
                            lhsT=w2b[:, dc * U + uc * 128 : dc * U + uc * 128 + 128],
                            rhs=vTb[:, dc * TK : (dc + 1) * TK],
                            start=(dc == 0),
                            stop=(dc == DC - 1),
                        )
                        if dc == DC - 1:
                            ins.then_inc(s_proj, 1)
                # scores: 2M*UC accumulating matmuls into one PSUM bank
                for qt, (m0, nm) in enumerate(KQ):
                    h = qt // 2
                    tensor.wait_ge(s_fold, 2 * h + 2)
                    tensor.wait_ge(s_trig, 4 + 2 * (qt + 1))
                    for ml in range(nm):
                        m = m0 + ml
                        for qmat, kmat in ((SqF, Ck), (CqF, Sk)):
                            for uc in range(UC):
                                ins = tensor.matmul(
                                    out=scores_ps,
                                    lhsT=qmat[:, (m * UC + uc) * 128 : (m * UC + uc + 1) * 128],
                                    rhs=kmat[:, (m * UC + uc) * TK : (m * UC + uc) * TK + TK],
                                    start=(qt == 0 and ml == 0 and qmat is SqF and uc == 0),
                                    stop=(qt == 3 and ml == nm - 1 and qmat is CqF and uc == UC - 1),
                                )
                ins.then_inc(s_mm, 1)
                # E transposes then context
                tensor.wait_ge(s_exp, 1)
                tensor.wait_ge(s_idb, 16)
                for sc in range(SC):
                    tensor.transpose(
                        out=tra_bf[:, sc * 128 : (sc + 1) * 128],
                        in_=E_sb[:, sc * 128 : (sc + 1) * 128],
                        identity=ident_bf[:, :],
                    ).then_inc(s_tra, 1)
                tensor.wait_ge(s_evt, 1)
                tensor.wait_ge(s_vbf, 16)
                for sc in range(SC):
                    ins = tensor.matmul(
                        out=ctx_ps,
                        lhsT=ET_sb[:, sc * 128 : (sc + 1) * 128],
                        rhs=v_bf[:, sc * DV : (sc + 1) * DV],
                        start=(sc == 0),
                        stop=(sc == SC - 1),
                    )
                ins.then_inc(s_ctx, 1)

    return nc


_NC = None


def _get_nc() -> bass.Bass:
    global _NC
    if _NC is None:
        _NC = build_bass()
    return _NC


_CONST = None


def make_in_maps(query, value, W1, W2, scale):
    global _CONST
    import ml_dtypes

    bf = ml_dtypes.bfloat16
    scale = np.asarray(scale, np.float32)
    if _CONST is None:
        _CONST = {"identb": np.eye(128).astype(bf)}
    bst = np.empty((128, M * UC * 128), np.float32)
    for m in range(M):
        for uc in range(UC):
            col = (m * UC + uc) * 128
            bst[:, col : col + 128] = (
                BS[m] * scale[uc * 128 : (uc + 1) * 128]
            )[:, None]
    bstab = bst.astype(bf)
    query = np.asarray(query, dtype=np.float32)
    value = np.asarray(value, dtype=np.float32)
    W1 = np.asarray(W1, np.float32)
    W2 = np.asarray(W2, np.float32)
    in_maps = []
    for c in range(N_CORES):
        b, th = c // 2, c % 2
        qloc = query[b, th * T_ROWS : (th + 1) * T_ROWS, :]
        vloc = value[b]
        pk = lambda a: np.ascontiguousarray(
            a.reshape(4, 128, a.shape[1]).transpose(1, 0, 2).reshape(128, -1)
        )
        in_maps.append(
            {
                "queryT": pk(qloc.T.astype(bf)),
                "valueT": pk(vloc.T.astype(bf)),
                "valuebf": pk(vloc.astype(bf)),
                "W1b": pk(W1.astype(bf)),
                "W2b": pk(W2.astype(bf)),
                "bstab": bstab,
                "identb": _CONST["identb"],
            }
        )
    return in_maps


def assemble(results):
    context = np.empty((B, TQ, DV), dtype=np.float32)
    attn = np.empty((B, TQ, TK), dtype=np.float32)
    for c in range(N_CORES):
        b, th = c // 2, c % 2
        context[b, th * T_ROWS : (th + 1) * T_ROWS, :] = results[c]["context"]
        attn[b, th * T_ROWS : (th + 1) * T_ROWS, :] = results[c]["attn"]
    return context, attn


def kernel(query, value, W1, W2, scale):
    nc = _get_nc()
    in_maps = make_in_maps(query, value, W1, W2, scale)
    res = run_bass_kernel_spmd(nc, in_maps, core_ids=list(range(N_CORES)))
    return assemble(res.results)


# revision 30
# speedup vs baseline: 1.1178x; 1.1178x over previous
"""Additive (Bahdanau) attention on 8 TRN2 NeuronCores (raw Bass).

Reference math (B=4, Tq=256, Tk=512, Dq=Dv=512, U=256):
    q = query @ W1; k = value @ W2
    scores[t,s] = sum_u scale[u] * tanh(q[t,u] + k[s,u])
    attn = softmax(scores, -1); context = attn @ value

Separable-sine reformulation: fit  tanh(z) ~= sum_m b_m sin(w_m z)
(M=8 free frequencies), then sin(w(q+k)) = sin(wq)cos(wk)+cos(wq)sin(wk):
    scores ~= sum_m (b_m scale_u sin(w_m q)) @ cos(w_m k)^T
            + (b_m scale_u cos(w_m q)) @ sin(w_m k)^T
i.e. 2M=16 rank-U matmuls.  The O(Tq*Tk*U) tanh tensor is never formed:
ACT evaluates sin only on the small q ([128,256]) / k ([512,256])
matrices.

The device Sin spline is accurate only for |arg| <~ 3.5, so arguments
are range-reduced per mode with a 2-op fp32 bit trick on DVE:
    u = z*(w/2pi) + 1536.625          (ts mult,add; exponent pinned
                                       to 2^10 so low 13 mantissa bits
                                       hold frac(u) * 2^13)
    w32 = (u & 0x1FFF) | 0x3F800000   (ts and,or; w32 in [1,2))
then the ACT's exact-FMA affine maps it back:
    sin(w z)  = Sin(2048pi * w32 - 2048pi - 5pi/4)
    cos(w z)  = Sin(... + pi/2)        args in [-pi-pi/4, pi-pi/4].

Softmax runs in [t_p, s] layout: exp with accum_out produces row sums
for free; attn needs no transpose; context uses 4 PE transposes of E.
Input DMAs are spread over all five engine queues (per-queue DMA
bandwidth ~45GB/s is the startup bottleneck).

Sharding: (b, tq-half) -> 8 cores, 128 query rows each; Tk local.
"""

from contextlib import ExitStack

import numpy as np

import concourse.bass as bass
import concourse.mybir as mybir
from concourse.bass_utils import run_bass_kernel_spmd

F32 = mybir.dt.float32
I32 = mybir.dt.int32
BF16 = mybir.dt.bfloat16
AF = mybir.ActivationFunctionType
OP = mybir.AluOpType

N_CORES = 8
B, TQ, TK, DQ, DV, U = 4, 256, 512, 512, 512, 256
T_ROWS = 128
UC = U // 128          # 2
DC = DQ // 128         # 4
SC = TK // 128         # 4
M = 8                  # sine modes
H = 2                  # mode halves
MH = M // H

WS = [0.15790899, 0.56623729, 1.04592589, 1.55170364,
      2.07477797, 2.60427305, 3.20631726, 4.24741697]
BS = [1.36630283, 0.45248371, 0.19916159, 0.09039594,
      0.04130632, 0.01723859, 0.01007287, 0.00330992]

SC2 = float(np.float32(1024 * 2 * np.pi))
BIAS_S = float(np.float32(-np.float64(np.float32(SC2)) - np.pi - np.pi / 4))
BIAS_C = float(np.float32(-np.float64(np.float32(SC2)) - np.pi + np.pi / 4))
OFFS = 1536.625


def build_bass() -> bass.Bass:
    nc = bass.Bass()
    qt_ext = nc.declare_dram_parameter("queryT", [128, DC * 128], BF16, isOutput=False)
    vt_ext = nc.declare_dram_parameter("valueT", [128, DC * TK], BF16, isOutput=False)
    vb_ext = nc.declare_dram_parameter("valuebf", [128, SC * DV], BF16, isOutput=False)
    w1_ext = nc.declare_dram_parameter("W1b", [128, DC * U], BF16, isOutput=False)
    w2_ext = nc.declare_dram_parameter("W2b", [128, DC * U], BF16, isOutput=False)
    bst_ext = nc.declare_dram_parameter("bstab", [128, M * UC * 128], BF16, isOutput=False)
    idb_ext = nc.declare_dram_parameter("identb", [128, 128], BF16, isOutput=False)
    ctx_ext = nc.declare_dram_parameter("context", [T_ROWS, DV], F32, isOutput=True)
    attn_ext = nc.declare_dram_parameter("attn", [T_ROWS, TK], F32, isOutput=True)

    es = ExitStack()
    with es:
        _n = [0]

        def sb(shape, dt):
            _n[0] += 1
            return es.enter_context(nc.sbuf_tensor(f"sb{_n[0]}", shape, dt))

        # ---- SBUF ----
        vTb = sb([128, DC * TK], BF16)          # [d_p, (dc, s)]
        qTb = sb([128, DC * 128], BF16)         # [d_p, (dc, t)]
        w1b = sb([128, DC * U], BF16)
        w2b = sb([128, DC * U], BF16)
        v_bf = sb([128, SC * DV], BF16)         # [s_p, (sc, d)]
        bs_full = sb([128, M * UC * 128], BF16)  # [u_p, (m, uc, t-bcast)]
        ident_bf = sb([128, 128], BF16)
        q_f = sb([128, UC * 128], F32)          # [u_p, (uc, t)]
        k_f = sb([128, UC * TK], F32)           # [u_p, (uc, s)]
        u_q = sb([128, M * UC * 128], F32)      # [u_p, (m, uc, t)]
        w_q = sb([128, M * UC * 128], F32)
        u_k = sb([128, M * UC * TK], F32)       # [u_p, (m, uc, s)]
        w_k = sb([128, M * UC * TK], F32)
        Sq = sb([128, M * UC * 128], BF16)
        Cq = sb([128, M * UC * 128], BF16)
        SqF = sb([128, M * UC * 128], BF16)     # folded with b_m*scale_u
        CqF = sb([128, M * UC * 128], BF16)
        Sk = sb([128, M * UC * TK], BF16)
        Ck = sb([128, M * UC * TK], BF16)
        E_sb = sb([128, TK], BF16)              # [t_p, s]
        ET_sb = sb([128, SC * 128], BF16)       # [s_p, (sc, t)]
        sums = sb([128, 1], F32)
        r_sb = sb([128, 1], F32)
        attn_sb = sb([128, TK], F32)
        ctx_sb = sb([128, DV], F32)
        bias_s = sb([128, 1], F32)
        bias_c = sb([128, 1], F32)
        bias_o = sb([128, 1], F32)
        scratch = sb([128, 1], F32)

        QW = UC * 128        # 256 free elems per mode, q side
        KW = UC * TK         # 1024 per mode, k side

        # ---- PSUM ----
        psA = es.enter_context(nc.psum_tensor("psA", [128, 2048], F32))
        psB = es.enter_context(nc.psum_tensor("psB", [128, 2048], F32))
        scores_ps = psA[:, 0:512]
        ctx_ps = psA[:, 512:1024]
        tra_ps = psA[:, 1024:1536]
        k_ps = [psB[:, 0:512], psB[:, 512:1024]]
        q_ps = [psB[:, 1024:1152], psB[:, 1536:1664]]
        q_ps_view = psB[:, 1024:2048].rearrange("p (uc x) -> p uc x", uc=2)[:, :, 0:128]
        tra_bf = tra_ps.bitcast(BF16)           # [128, 1024] bf16

        sem = lambda name: es.enter_context(nc.semaphore(name))
        s_qt = sem("s_qt")
        s_w1a = sem("s_w1a")
        s_w1b = sem("s_w1b")
        s_w2a = sem("s_w2a")
        s_w2b = sem("s_w2b")
        s_vt = [sem(f"s_vt{i}") for i in range(DC)]
        s_vbf = sem("s_vbf")
        s_idb = sem("s_idb")
        s_bst = sem("s_bst")
        s_c = sem("s_c")
        s_proj = sem("s_proj")   # q0,q1,k0,k1
        s_evq = sem("s_evq")
        s_evk = sem("s_evk")
        s_uq = sem("s_uq")
        s_uk = sem("s_uk")
        s_ua = sem("s_ua")
        s_yq = sem("s_yq")       # w_q halves ready
        s_yk = sem("s_yk")
        s_trig = sem("s_trig")   # qh0 s,c qh1 s,c kh0 s,c kh1 s,c
        s_fold = sem("s_fold")   # h0 S,C h1 S,C
        s_mm = sem("s_mm")
        s_exp = sem("s_exp")
        s_tra = sem("s_tra")
        s_evt = sem("s_evt")
        s_ctx = sem("s_ctx")
        s_o = sem("s_o")
        s_dout = sem("s_dout")
        s_dout2 = sem("s_dout2")

        def ts1(vector, out_t, in_t, m, width):
            return vector.tensor_scalar(
                out=out_t[:, m * width : (m + 1) * width],
                in0=in_t[:, :],
                scalar1=float(WS[m] / (2 * np.pi)),
                scalar2=OFFS,
                op0=OP.mult,
                op1=OP.add,
            )

        def ts2(vector, out_t, in_t, m0, nm, width):
            sl = slice(m0 * width, (m0 + nm) * width)
            return vector.tensor_scalar(
                out=out_t[:, sl].bitcast(I32),
                in0=in_t[:, sl].bitcast(I32),
                scalar1=0x00001FFF,
                scalar2=0x3F800000,
                op0=OP.bitwise_and,
                op1=OP.bitwise_or,
            )

        with nc.Block() as block:

            @block.sync
            def _(sync):
                sync.dma_start(out=qTb[:, :], in_=qt_ext[:, :]).then_inc(s_qt, 16)
                sync.dma_start(
                    out=vTb[:, 0:TK], in_=vt_ext[:, 0:TK]
                ).then_inc(s_vt[0], 16)
                sync.dma_start(
                    out=vTb[:, 1 * TK : 2 * TK], in_=vt_ext[:, 1 * TK : 2 * TK]
                ).then_inc(s_vt[1], 16)
                sync.wait_ge(s_o, 1)
                sync.dma_start(out=attn_ext[:, 0:256], in_=attn_sb[:, 0:256]).then_inc(s_dout, 16)
                sync.wait_ge(s_o, 2)
                sync.dma_start(out=ctx_ext[:, 256:512], in_=ctx_sb[:, 256:512]).then_inc(s_dout, 16)
                sync.wait_ge(s_dout, 48)
                sync.wait_ge(s_dout2, 16)

            @block.gpsimd
            def _(gpsimd):
                gpsimd.dma_start(
                    out=w1b[:, 2 * U : 4 * U], in_=w1_ext[:, 2 * U : 4 * U]
                ).then_inc(s_w1b, 16)
                gpsimd.dma_start(
                    out=w2b[:, 2 * U : 4 * U], in_=w2_ext[:, 2 * U : 4 * U]
                ).then_inc(s_w2b, 16)
                gpsimd.dma_start(
                    out=vTb[:, 3 * TK : 4 * TK], in_=vt_ext[:, 3 * TK : 4 * TK]
                ).then_inc(s_vt[3], 16)
                gpsimd.dma_start(out=bs_full[:, :], in_=bst_ext[:, :]).then_inc(s_bst, 16)
                gpsimd.dma_start(out=v_bf[:, :], in_=vb_ext[:, :]).then_inc(s_vbf, 16)
                gpsimd.dma_start(out=ident_bf[:, :], in_=idb_ext[:, :]).then_inc(s_idb, 16)
                gpsimd.wait_ge(s_o, 2)
                gpsimd.dma_start(out=ctx_ext[:, 0:256], in_=ctx_sb[:, 0:256]).then_inc(s_dout2, 16)

            @block.vector
            def _(vector):
                vector.memset(bias_s[:, :], BIAS_S)
                vector.memset(bias_o[:, :], OFFS)
                vector.memset(bias_c[:, :], BIAS_C).then_inc(s_c, 1)
                # q reductions, per half: ts1 x4 then ts2 (self-sems order
                # same-engine RAW for the race model; ~free on the queue)
                vector.wait_ge(s_evq, 1)
                for h in range(H):
                    for ml in range(MH):
                        ins = ts1(vector, u_q, q_f, h * MH + ml, QW)
                    ins.then_inc(s_uq, 1)
                    vector.wait_ge(s_uq, h + 1)
                    ts2(vector, w_q, u_q, h * MH, MH, QW).then_inc(s_yq, 1)
                # k reductions, quarter-granular (2 modes per ts2/trig group)
                vector.wait_ge(s_evk, 1)
                for qt in range(4):
                    if qt < 3:
                        for ml in range(2):
                            ins = ts1(vector, u_k, k_f, qt * 2 + ml, KW)
                        ins.then_inc(s_uk, 1)
                        vector.wait_ge(s_uk, qt + 1)
                    else:
                        # modes 6/7 ts1 done by ACT (Identity assist)
                        vector.wait_ge(s_ua, 2)
                    ts2(vector, w_k, u_k, qt * 2, 2, KW).then_inc(s_yk, 1)
                # folds: SqF/CqF = Sq/Cq * (b_m scale_u), full-size table
                vector.wait_ge(s_bst, 16)
                for h in range(H):
                    vector.wait_ge(s_trig, 2 * h + 2)
                    sl = slice(h * MH * QW, (h + 1) * MH * QW)
                    for src, dst in ((Sq, SqF), (Cq, CqF)):
                        vector.tensor_tensor(
                            out=dst[:, sl], in0=src[:, sl], in1=bs_full[:, sl],
                            op=OP.mult,
                        ).then_inc(s_fold, 1)
                # epilogue
                vector.wait_ge(s_exp, 1)
                vector.reciprocal(out=r_sb[:, :], in_=sums[:, :])
                vector.drain()
                vector.tensor_scalar_mul(
                    out=attn_sb[:, :], in0=E_sb[:, :], scalar1=r_sb[:, 0:1]
                ).then_inc(s_o, 1)
                vector.wait_ge(s_ctx, 1)
                vector.tensor_scalar_mul(
                    out=ctx_sb[:, :], in0=ctx_ps, scalar1=r_sb[:, 0:1]
                ).then_inc(s_o, 1)

            @block.scalar
            def _(scalar):
                scalar.dma_start(
                    out=w1b[:, 0 : 2 * U], in_=w1_ext[:, 0 : 2 * U]
                ).then_inc(s_w1a, 16)
                scalar.dma_start(
                    out=w2b[:, 0 : 2 * U], in_=w2_ext[:, 0 : 2 * U]
                ).then_inc(s_w2a, 16)
                scalar.dma_start(
                    out=vTb[:, 2 * TK : 3 * TK], in_=vt_ext[:, 2 * TK : 3 * TK]
                ).then_inc(s_vt[2], 16)
                # dummy sin pulls the trig table load off the critical path
                scalar.wait_ge(s_c, 1)
                scalar.activation(out=scratch[:, :], in_=bias_s[:, :], func=AF.Sin)
                # q evac
                scalar.wait_ge(s_proj, 2)
                scalar.copy(
                    out=q_f[:, :].rearrange("p (uc t) -> p uc t", uc=2),
                    in_=q_ps_view,
                ).then_inc(s_evq, 1)
                # q trig
                for h in range(H):
                    qs = slice(h * MH * QW, (h + 1) * MH * QW)
                    scalar.wait_ge(s_yq, h + 1)
                    scalar.activation(out=Sq[:, qs], in_=w_q[:, qs], func=AF.Sin,
                                      scale=SC2, bias=bias_s[:, 0:1]).then_inc(s_trig, 1)
                    scalar.activation(out=Cq[:, qs], in_=w_q[:, qs], func=AF.Sin,
                                      scale=SC2, bias=bias_c[:, 0:1]).then_inc(s_trig, 1)
                # k evac
                scalar.wait_ge(s_proj, 4)
                scalar.copy(out=k_f[:, :], in_=psB[:, 0:1024]).then_inc(s_evk, 1)
                for m in (6, 7):
                    scalar.activation(out=u_k[:, m * KW : (m + 1) * KW],
                                      in_=k_f[:, :], func=AF.Identity,
                                      scale=float(WS[m] / (2 * np.pi)),
                                      bias=bias_o[:, 0:1]).then_inc(s_ua, 1)
                # k trig, quarter-granular
                for qt in range(4):
                    ks = slice(qt * 2 * KW, (qt + 1) * 2 * KW)
                    scalar.wait_ge(s_yk, qt + 1)
                    scalar.activation(out=Sk[:, ks], in_=w_k[:, ks], func=AF.Sin,
                                      scale=SC2, bias=bias_s[:, 0:1]).then_inc(s_trig, 1)
                    scalar.activation(out=Ck[:, ks], in_=w_k[:, ks], func=AF.Sin,
                                      scale=SC2, bias=bias_c[:, 0:1]).then_inc(s_trig, 1)
                # dummy exp: pull the exp table load off the critical path
                scalar.activation(out=scratch[:, :], in_=bias_s[:, :], func=AF.Exp)
                # softmax exp with free row sums
                scalar.wait_ge(s_mm, 1)
                scalar.activation(out=E_sb[:, :], in_=scores_ps, func=AF.Exp,
                                  accum_out=sums[:, 0:1]).then_inc(s_exp, 1)
                # ET evac for the context matmuls
                scalar.wait_ge(s_tra, 4)
                scalar.copy(out=ET_sb[:, :], in_=tra_bf[:, 0 : SC * 128]).then_inc(s_evt, 1)
                # attn second half on this queue
                scalar.wait_ge(s_o, 1)
                scalar.dma_start(out=attn_ext[:, 256:512], in_=attn_sb[:, 256:512]).then_inc(s_dout, 16)

            @block.tensor
            def _(tensor):
                # q projection, dc-pipelined
                tensor.wait_ge(s_qt, 16)
                for dc in range(DC):
                    tensor.wait_ge(s_w1a if dc < 2 else s_w1b, 16)
                    for uc in range(UC):
                        ins = tensor.matmul(
                            out=q_ps[uc],
                            lhsT=w1b[:, dc * U + uc * 128 : dc * U + uc * 128 + 128],
                            rhs=qTb[:, dc * 128 : (dc + 1) * 128],
                            start=(dc == 0),
                            stop=(dc == DC - 1),
                        )
                        if dc == DC - 1:
                            ins.then_inc(s_proj, 1)
                # k projection, dc-pipelined
                for dc in range(DC):
                    tensor.wait_ge(s_w2a if dc < 2 else s_w2b, 16)
                    tensor.wait_ge(s_vt[dc], 16)
                    for uc in range(UC):
                        ins = tensor.matmul(
                            out=k_ps[uc],
                            lhsT=w2b[:, dc * U + uc * 128 : dc * U + uc * 128 + 128],
                            rhs=vTb[:, dc * TK : (dc + 1) * TK],
                            start=(dc == 0),
                            stop=(dc == DC - 1),
                        )
                        if dc == DC - 1:
                            ins.then_inc(s_proj, 1)
                # scores: 2M*UC accumulating matmuls into one PSUM bank
                for qt in range(4):
                    h = qt // 2
                    tensor.wait_ge(s_fold, 2 * h + 2)
                    tensor.wait_ge(s_trig, 4 + 2 * (qt + 1))
                    for ml in range(2):
                        m = qt * 2 + ml
                        for qmat, kmat in ((SqF, Ck), (CqF, Sk)):
                            for uc in range(UC):
                                ins = tensor.matmul(
                                    out=scores_ps,
                                    lhsT=qmat[:, (m * UC + uc) * 128 : (m * UC + uc + 1) * 128],
                                    rhs=kmat[:, (m * UC + uc) * TK : (m * UC + uc) * TK + TK],
                                    start=(qt == 0 and ml == 0 and qmat is SqF and uc == 0),
                                    stop=(qt == 3 and ml == 1 and qmat is CqF and uc == UC - 1),
                                )
                ins.then_inc(s_mm, 1)
                # E transposes then context
                tensor.wait_ge(s_exp, 1)
                tensor.wait_ge(s_idb, 16)
                for sc in range(SC):
                    tensor.transpose(
                        out=tra_bf[:, sc * 128 : (sc + 1) * 128],
                        in_=E_sb[:, sc * 128 : (sc + 1) * 128],
                        identity=ident_bf[:, :],
                    ).then_inc(s_tra, 1)
                tensor.wait_ge(s_evt, 1)
                tensor.wait_ge(s_vbf, 16)
                for sc in range(SC):
                    ins = tensor.matmul(
                        out=ctx_ps,
                        lhsT=ET_sb[:, sc * 128 : (sc + 1) * 128],
                        rhs=v_bf[:, sc * DV : (sc + 1) * DV],
                        start=(sc == 0),
                        stop=(sc == SC - 1),
                    )
                ins.then_inc(s_ctx, 1)

    return nc


_NC = None


def _get_nc() -> bass.Bass:
    global _NC
    if _NC is None:
        _NC = build_bass()
    return _NC


_CONST = None


def make_in_maps(query, value, W1, W2, scale):
    global _CONST
    import ml_dtypes

    bf = ml_dtypes.bfloat16
    scale = np.asarray(scale, np.float32)
    if _CONST is None:
        _CONST = {"identb": np.eye(128).astype(bf)}
    bst = np.empty((128, M * UC * 128), np.float32)
    for m in range(M):
        for uc in range(UC):
            col = (m * UC + uc) * 128
            bst[:, col : col + 128] = (
                BS[m] * scale[uc * 128 : (uc + 1) * 128]
            )[:, None]
    bstab = bst.astype(bf)
    query = np.asarray(query, dtype=np.float32)
    value = np.asarray(value, dtype=np.float32)
    W1 = np.asarray(W1, np.float32)
    W2 = np.asarray(W2, np.float32)
    in_maps = []
    for c in range(N_CORES):
        b, th = c // 2, c % 2
        qloc = query[b, th * T_ROWS : (th + 1) * T_ROWS, :]
        vloc = value[b]
        pk = lambda a: np.ascontiguousarray(
            a.reshape(4, 128, a.shape[1]).transpose(1, 0, 2).reshape(128, -1)
        )
        in_maps.append(
            {
                "queryT": pk(qloc.T.astype(bf)),
                "valueT": pk(vloc.T.astype(bf)),
                "valuebf": pk(vloc.astype(bf)),
                "W1b": pk(W1.astype(bf)),
                "W2b": pk(W2.astype(bf)),
                "bstab": bstab,
                "identb": _CONST["identb"],
            }
        )
    return in_maps


def assemble(results):
    context = np.empty((B, TQ, DV), dtype=np.float32)
    attn = np.empty((B, TQ, TK), dtype=np.float32)
    for c in range(N_CORES):
        b, th = c // 2, c % 2
        context[b, th * T_ROWS : (th + 1) * T_ROWS, :] = results[c]["context"]
        attn[b, th * T_ROWS : (th + 1) * T_ROWS, :] = results[c]["attn"]
    return context, attn


def kernel(query, value, W1, W2, scale):
    nc = _get_nc()
    in_maps = make_in_maps(query, value, W1, W2, scale)
    res = run_bass_kernel_spmd(nc, in_maps, core_ids=list(range(N_CORES)))
    return assemble(res.results)
